# revision 7
# baseline (speedup 1.0000x reference)
"""LiteLinear (dense linear + per-token LoRA adapters) on 8 Trainium2 cores.

Sharding: data-parallel over tokens. Each core computes 1024 tokens:
  out = x @ W^T + bias + per-token LoRA delta.

Device kernel (per core), all matmuls in float32r (full-rate fp32 mode):
  - Computes out^T [D_OUT x TOK]; host transposes back on assembly.
  - Stationary operand = W^T sub-chunk [128d x 128o], moving = x^T
    [128d x 512tok] -> each weight load serves 2 matmuls.
  - x^T resident in SBUF (128KB/partition); W^T streamed exactly once as
    4-k-chunk quad DMAs (fewer sequencer dma_starts). Startup is k-major
    interleaved so PE trickles h-matmuls + first-group matmuls while x
    streams in.
  - o-groups of [3,1,4,4,4,4,4,4,4] x128 outputs; psum = width x 2
    token-halves banks; the 3-wide first group leaves 2 banks for h.
  - Output DMAs issue from gpsimd, consts/lora_a from scalar, x/W from
    sync: three independent issue queues, so W prefetch never queues
    behind evictions.
  - LoRA: h^T = A_cat @ x^T, masked+scaled by one DVE multiply with a
    host-built maskT (folds scalings + one-hot); delta enters each
    out-tile as one extra accumulating matmul (lhsT=B_cat chunk,
    rhs=hmask^T). Bias folded into eviction via tensor_scalar_add.
"""

import numpy as np

import sys

if "/opt/trn_rl_repo" not in sys.path:
    sys.path.insert(0, "/opt/trn_rl_repo")

import concourse.bass as bass
import concourse.mybir as mybir
import concourse.tile as tile
from concourse import bacc
from concourse.bass_utils import run_bass_kernel_spmd

N_TOK = 8192
D_IN = 4096
D_OUT = 4096
N_ADAPTERS = 8
RANK = 16
AR = N_ADAPTERS * RANK  # 128
N_CORES = 8
TOK = N_TOK // N_CORES  # 1024 tokens per core

P = 128            # partitions
FREE = 512         # matmul moving free dim (fp32 max, == 1 PSUM bank)
KC = D_IN // P     # 32 contraction chunks
KQ = 4             # k-chunks per W/A quad DMA
TH = TOK // FREE   # 2 token halves
GROUPS = [3, 1, 4, 4, 4, 4, 4, 4, 4]  # o128-tiles per group (sum 32)

F32 = mybir.dt.float32
F32R = mybir.dt.float32r

_CACHE = {}


def _quad_src(dram_ap, row0, ncols, col0, width, row_stride):
    """3D AP reading KQ consecutive [P x width] chunks: partition=row,
    then (chunk, col). row_stride = DRAM row length in elements."""
    return bass.AP(
        tensor=dram_ap.tensor,
        offset=dram_ap.offset + row0 * row_stride + col0,
        ap=[[row_stride, P], [P * row_stride, KQ], [1, width]],
    )


def _build_nc():
    nc = bacc.Bacc(None, target_bir_lowering=False, debug=True)

    xT = nc.dram_tensor("xT", [D_IN, TOK], F32R, kind="ExternalInput")
    wT = nc.dram_tensor("wT", [D_IN, D_OUT], F32R, kind="ExternalInput")
    aT = nc.dram_tensor("aT", [D_IN, AR], F32R, kind="ExternalInput")
    bcat = nc.dram_tensor("bcat", [AR, D_OUT], F32R, kind="ExternalInput")
    maskT = nc.dram_tensor("maskT", [AR, TOK], F32, kind="ExternalInput")
    biasr = nc.dram_tensor("biasr", [P, D_OUT // P], F32, kind="ExternalInput")
    outT = nc.dram_tensor("outT", [D_OUT, TOK], F32, kind="ExternalOutput")

    with tile.TileContext(nc) as tc:
        with (
            tc.tile_pool(name="xpool", bufs=1) as xpool,
            tc.tile_pool(name="const", bufs=1) as const,
            tc.tile_pool(name="wpool", bufs=3) as wpool,
            tc.tile_pool(name="apool", bufs=2) as apool,
            tc.tile_pool(name="opool", bufs=3) as opool,
            tc.tile_pool(name="psum", bufs=8, space="PSUM") as psum,
        ):
            # small consts on the scalar issue queue
            biasr_sb = const.tile([P, D_OUT // P], F32, tag="biasr")
            nc.scalar.dma_start(out=biasr_sb[:], in_=biasr[:, :])
            maskT_sb = const.tile([P, TOK], F32, tag="maskT")
            nc.scalar.dma_start(out=maskT_sb[:], in_=maskT[:, :])
            bcat_sb = const.tile([P, D_OUT], F32R, tag="bcat")
            nc.scalar.dma_start(out=bcat_sb[:], in_=bcat[:, :])

            hmask = const.tile([P, TOK], F32R, tag="hmask")

            # ---- startup: k-major interleaved x / lora_a / W(group0) ----
            G0 = GROUPS[0]
            ph = [
                psum.tile([P, FREE], F32, tag="ps", name=f"ph_{t}")
                for t in range(TH)
            ]
            pg = [
                psum.tile([P, FREE], F32, tag="ps", name=f"pg0_{i}")
                for i in range(G0 * TH)
            ]
            xt = []
            at = wt = None
            for k in range(KC):
                t = xpool.tile([P, TOK], F32R, tag=f"xt{k}", name=f"xt{k}")
                nc.sync.dma_start(out=t[:], in_=xT[k * P:(k + 1) * P, :])
                xt.append(t)
                if k % KQ == 0:
                    at = apool.tile([P, KQ * AR], F32R, tag="at",
                                    name=f"at{k}")
                    nc.scalar.dma_start(
                        out=at[:], in_=_quad_src(aT[:], k * P, AR, 0, AR, AR))
                    wt = wpool.tile([P, KQ * G0 * P], F32R, tag="wt",
                                    name=f"wt0_{k}")
                    nc.sync.dma_start(
                        out=wt[:],
                        in_=_quad_src(wT[:], k * P, G0 * P, 0, G0 * P, D_OUT))
                kk = k % KQ
                for th in range(TH):
                    tsl = slice(th * FREE, (th + 1) * FREE)
                    nc.tensor.matmul(
                        ph[th][:], at[:, kk * AR:(kk + 1) * AR], xt[k][:, tsl],
                        start=(k == 0), stop=(k == KC - 1),
                    )
                    for j in range(G0):
                        nc.tensor.matmul(
                            pg[j * TH + th][:],
                            wt[:, (kk * G0 + j) * P:(kk * G0 + j + 1) * P],
                            xt[k][:, tsl],
                            start=(k == 0), stop=False,
                        )

            # h -> hmask (scaled, masked)
            for th in range(TH):
                tsl = slice(th * FREE, (th + 1) * FREE)
                nc.vector.tensor_mul(hmask[:, tsl], ph[th][:], maskT_sb[:, tsl])

            # ---- per-group: delta matmul + eviction (out DMA on gpsimd) ----
            def finish_group(pg, ooff, width):
                for j in range(width):
                    for th in range(TH):
                        tsl = slice(th * FREE, (th + 1) * FREE)
                        nc.tensor.matmul(
                            pg[j * TH + th][:],
                            bcat_sb[:, ooff + j * P:ooff + (j + 1) * P],
                            hmask[:, tsl],
                            start=False, stop=True,
                        )
                for j in range(width):
                    om = ooff // P + j
                    ob = opool.tile([P, TOK], F32, tag="ob", name=f"ob_{om}")
                    for th in range(TH):
                        tsl = slice(th * FREE, (th + 1) * FREE)
                        nc.vector.tensor_scalar_add(
                            ob[:, tsl], pg[j * TH + th][:],
                            biasr_sb[:, om:om + 1],
                        )
                    nc.gpsimd.dma_start(
                        out=outT[ooff + j * P:ooff + (j + 1) * P, :], in_=ob[:]
                    )

            finish_group(pg, 0, G0)

            ooff = G0 * P
            for g, width in enumerate(GROUPS[1:], start=1):
                pg = [
                    psum.tile([P, FREE], F32, tag="ps", name=f"pg{g}_{i}")
                    for i in range(width * TH)
                ]
                for k in range(KC):
                    if k % KQ == 0:
                        wt = wpool.tile([P, KQ * width * P], F32R, tag="wt",
                                        name=f"wt{g}_{k}")
                        nc.sync.dma_start(
                            out=wt[:],
                            in_=_quad_src(wT[:], k * P, width * P, ooff,
                                          width * P, D_OUT))
                    kk = k % KQ
                    for j in range(width):
                        for th in range(TH):
                            tsl = slice(th * FREE, (th + 1) * FREE)
                            nc.tensor.matmul(
                                pg[j * TH + th][:],
                                wt[:, (kk * width + j) * P:
                                   (kk * width + j + 1) * P],
                                xt[k][:, tsl],
                                start=(k == 0), stop=False,
                            )
                finish_group(pg, ooff, width)
                ooff += width * P

    nc.compile()
    return nc


def _prep_inputs(x, weight, bias, lora_a, lora_b, scalings, lora_mapping):
    x = np.ascontiguousarray(x, dtype=np.float32)
    weight = np.ascontiguousarray(weight, dtype=np.float32)
    bias = np.ascontiguousarray(bias, dtype=np.float32)
    lora_a = np.ascontiguousarray(lora_a, dtype=np.float32)
    lora_b = np.ascontiguousarray(lora_b, dtype=np.float32)
    scalings = np.ascontiguousarray(scalings, dtype=np.float32)
    lora_mapping = np.asarray(lora_mapping)

    xT = np.ascontiguousarray(x.T)                                   # [D_IN, N_TOK]
    wT = np.ascontiguousarray(weight.T)                              # [D_IN, D_OUT]
    aT = np.ascontiguousarray(
        lora_a.transpose(2, 0, 1).reshape(D_IN, AR))                 # [D_IN, (a r)]
    bcat = np.ascontiguousarray(
        lora_b.transpose(0, 2, 1).reshape(AR, D_OUT))                # [(a r), D_OUT]
    # biasr[p, m] = bias[m*128 + p]
    biasr = np.ascontiguousarray(bias.reshape(D_OUT // P, P).T)      # [P, 32]
    # maskT[(a r), n] = scalings[a] * (lora_mapping[n] == a+1)
    ids = np.arange(1, N_ADAPTERS + 1, dtype=lora_mapping.dtype)
    onehot = (lora_mapping[None, :] == ids[:, None]).astype(np.float32)  # [A, N]
    maskT = (onehot * scalings[:, None]).repeat(RANK, axis=0)        # [(a r), N]
    maskT = np.ascontiguousarray(maskT)

    in_maps = []
    for c in range(N_CORES):
        tsl = slice(c * TOK, (c + 1) * TOK)
        in_maps.append({
            "xT": np.ascontiguousarray(xT[:, tsl]),
            "wT": wT,
            "aT": aT,
            "bcat": bcat,
            "maskT": np.ascontiguousarray(maskT[:, tsl]),
            "biasr": biasr,
        })
    return in_maps


def run(inputs, trace=False):
    if "nc" not in _CACHE:
        _CACHE["nc"] = _build_nc()
    nc = _CACHE["nc"]
    in_maps = _prep_inputs(**inputs)
    res = run_bass_kernel_spmd(
        nc, in_maps, list(range(N_CORES)), trace=trace,
    )
    out = np.concatenate(
        [np.ascontiguousarray(r["outT"].T) for r in res.results], axis=0
    )
    return out, res


def kernel(**inputs) -> np.ndarray:
    out, _ = run(inputs, trace=False)
    return out


# revision 8
# speedup vs baseline: 1.0346x; 1.0346x over previous
"""LiteLinear (dense linear + per-token LoRA adapters) on 8 Trainium2 cores.

Sharding: data-parallel over tokens. Each core computes 1024 tokens:
  out = x @ W^T + bias + per-token LoRA delta.

Device kernel (per core), all matmuls in float32r (full-rate fp32 mode):
  - Computes out^T [D_OUT x TOK]; host transposes back on assembly.
  - Stationary operand = W^T sub-chunk [128d x 128o], moving = x^T
    [128d x 512tok]. x^T resident in SBUF (128KB/partition); W^T
    streamed exactly once.
  - W^T and A_cat^T are re-laid-out on the host in quad-major form so
    each streamed DMA covers 4 contraction chunks with 2-8KB contiguous
    lines (few, fat descriptors; one dma_start per 4 k-chunks).
  - Startup is k-major interleaved: PE trickles h-matmuls + first-group
    matmuls while x streams in.
  - o-groups of [3,1,4,4,4,4,4,4,4] x128 outputs; psum = width x 2
    token-halves banks; the 3-wide first group leaves 2 banks for h.
  - Output DMAs issue from gpsimd, lora/consts from scalar, x/W from
    sync: independent issue queues so W prefetch never queues behind
    evictions.
  - LoRA: h^T = A_cat @ x^T, masked+scaled by one DVE multiply with a
    host-built maskT (folds scalings + one-hot); delta enters each
    out-tile as one extra accumulating matmul (lhsT=B_cat chunk,
    rhs=hmask^T). Bias folded into eviction via tensor_scalar_add.
"""

import numpy as np

import sys

if "/opt/trn_rl_repo" not in sys.path:
    sys.path.insert(0, "/opt/trn_rl_repo")

import concourse.bass as bass
import concourse.mybir as mybir
import concourse.tile as tile
from concourse import bacc
from concourse.bass_utils import run_bass_kernel_spmd

N_TOK = 8192
D_IN = 4096
D_OUT = 4096
N_ADAPTERS = 8
RANK = 16
AR = N_ADAPTERS * RANK  # 128
N_CORES = 8
TOK = N_TOK // N_CORES  # 1024 tokens per core

P = 128            # partitions
FREE = 512         # matmul moving free dim (fp32 max, == 1 PSUM bank)
KC = D_IN // P     # 32 contraction chunks
KQ = 4             # k-chunks per W/A quad DMA
NQ = KC // KQ      # 8 quads
TH = TOK // FREE   # 2 token halves
GROUPS = [3, 1, 4, 4, 4, 4, 4, 4, 4]  # o128-tiles per group (sum 32)

F32 = mybir.dt.float32
F32R = mybir.dt.float32r

_CACHE = {}


def _build_nc():
    nc = bacc.Bacc(None, target_bir_lowering=False, debug=True)

    xT = nc.dram_tensor("xT", [D_IN, TOK], F32R, kind="ExternalInput")
    # quad-major W: [kq, p, (g kk cols_g)] with per-group contiguous blocks
    wTr = nc.dram_tensor("wTr", [NQ, P, KQ * D_OUT], F32R,
                         kind="ExternalInput")
    # quad-major A_cat: [kq, p, (kk ar)]
    aTq = nc.dram_tensor("aTq", [NQ, P, KQ * AR], F32R, kind="ExternalInput")
    bcat = nc.dram_tensor("bcat", [AR, D_OUT], F32R, kind="ExternalInput")
    maskT = nc.dram_tensor("maskT", [AR, TOK], F32, kind="ExternalInput")
    biasr = nc.dram_tensor("biasr", [P, D_OUT // P], F32, kind="ExternalInput")
    outT = nc.dram_tensor("outT", [D_OUT, TOK], F32, kind="ExternalOutput")

    def w_quad_src(kq, goff, blk):
        return bass.AP(
            tensor=wTr[:].tensor,
            offset=kq * P * KQ * D_OUT + goff,
            ap=[[KQ * D_OUT, P], [1, blk]],
        )

    def a_quad_src(kq):
        return bass.AP(
            tensor=aTq[:].tensor,
            offset=kq * P * KQ * AR,
            ap=[[KQ * AR, P], [1, KQ * AR]],
        )

    with tile.TileContext(nc) as tc:
        with (
            tc.tile_pool(name="xpool", bufs=1) as xpool,
            tc.tile_pool(name="const", bufs=1) as const,
            tc.tile_pool(name="wpool", bufs=3) as wpool,
            tc.tile_pool(name="apool", bufs=2) as apool,
            tc.tile_pool(name="opool", bufs=3) as opool,
            tc.tile_pool(name="psum", bufs=8, space="PSUM") as psum,
        ):
            hmask = const.tile([P, TOK], F32R, tag="hmask")
            biasr_sb = const.tile([P, D_OUT // P], F32, tag="biasr")
            maskT_sb = const.tile([P, TOK], F32, tag="maskT")
            bcat_sb = const.tile([P, D_OUT], F32R, tag="bcat")

            # ---- startup: k-major interleaved x / lora_a / W(group0) ----
            G0 = GROUPS[0]
            ph = [
                psum.tile([P, FREE], F32, tag="ps", name=f"ph_{t}")
                for t in range(TH)
            ]
            pg = [
                psum.tile([P, FREE], F32, tag="ps", name=f"pg0_{i}")
                for i in range(G0 * TH)
            ]
            xt = []
            at = wt = None
            for k in range(KC):
                t = xpool.tile([P, TOK], F32R, tag=f"xt{k}", name=f"xt{k}")
                nc.sync.dma_start(out=t[:], in_=xT[k * P:(k + 1) * P, :])
                xt.append(t)
                if k % KQ == 0:
                    kq = k // KQ
                    at = apool.tile([P, KQ * AR], F32R, tag="at",
                                    name=f"at{k}")
                    nc.scalar.dma_start(out=at[:], in_=a_quad_src(kq))
                    wt = wpool.tile([P, KQ * G0 * P], F32R, tag="wt",
                                    name=f"wt0_{k}")
                    nc.sync.dma_start(out=wt[:],
                                      in_=w_quad_src(kq, 0, KQ * G0 * P))
                if k == 1:
                    nc.scalar.dma_start(out=biasr_sb[:], in_=biasr[:, :])
                    nc.scalar.dma_start(out=maskT_sb[:], in_=maskT[:, :])
                if k == 8:
                    nc.scalar.dma_start(out=bcat_sb[:], in_=bcat[:, :])
                kk = k % KQ
                for th in range(TH):
                    tsl = slice(th * FREE, (th + 1) * FREE)
                    nc.tensor.matmul(
                        ph[th][:], at[:, kk * AR:(kk + 1) * AR], xt[k][:, tsl],
                        start=(k == 0), stop=(k == KC - 1),
                    )
                    for j in range(G0):
                        nc.tensor.matmul(
                            pg[j * TH + th][:],
                            wt[:, (kk * G0 + j) * P:(kk * G0 + j + 1) * P],
                            xt[k][:, tsl],
                            start=(k == 0), stop=False,
                        )

            # h -> hmask (scaled, masked)
            for th in range(TH):
                tsl = slice(th * FREE, (th + 1) * FREE)
                nc.vector.tensor_mul(hmask[:, tsl], ph[th][:], maskT_sb[:, tsl])

            # ---- per-group: delta matmul + eviction (out DMA on gpsimd) ----
            def finish_group(pg, ooff, width):
                for j in range(width):
                    for th in range(TH):
                        tsl = slice(th * FREE, (th + 1) * FREE)
                        nc.tensor.matmul(
                            pg[j * TH + th][:],
                            bcat_sb[:, ooff + j * P:ooff + (j + 1) * P],
                            hmask[:, tsl],
                            start=False, stop=True,
                        )
                for j in range(width):
                    om = ooff // P + j
                    ob = opool.tile([P, TOK], F32, tag="ob", name=f"ob_{om}")
                    for th in range(TH):
                        tsl = slice(th * FREE, (th + 1) * FREE)
                        nc.vector.tensor_scalar_add(
                            ob[:, tsl], pg[j * TH + th][:],
                            biasr_sb[:, om:om + 1],
                        )
                    nc.gpsimd.dma_start(
                        out=outT[ooff + j * P:ooff + (j + 1) * P, :], in_=ob[:]
                    )

            finish_group(pg, 0, G0)

            ooff = G0 * P
            for g, width in enumerate(GROUPS[1:], start=1):
                pg = [
                    psum.tile([P, FREE], F32, tag="ps", name=f"pg{g}_{i}")
                    for i in range(width * TH)
                ]
                for k in range(KC):
                    if k % KQ == 0:
                        wt = wpool.tile([P, KQ * width * P], F32R, tag="wt",
                                        name=f"wt{g}_{k}")
                        nc.sync.dma_start(
                            out=wt[:],
                            in_=w_quad_src(k // KQ, KQ * ooff, KQ * width * P))
                    kk = k % KQ
                    for j in range(width):
                        for th in range(TH):
                            tsl = slice(th * FREE, (th + 1) * FREE)
                            nc.tensor.matmul(
                                pg[j * TH + th][:],
                                wt[:, (kk * width + j) * P:
                                   (kk * width + j + 1) * P],
                                xt[k][:, tsl],
                                start=(k == 0), stop=False,
                            )
                finish_group(pg, ooff, width)
                ooff += width * P

    nc.compile()
    return nc


def _prep_inputs(x, weight, bias, lora_a, lora_b, scalings, lora_mapping):
    x = np.ascontiguousarray(x, dtype=np.float32)
    weight = np.ascontiguousarray(weight, dtype=np.float32)
    bias = np.ascontiguousarray(bias, dtype=np.float32)
    lora_a = np.ascontiguousarray(lora_a, dtype=np.float32)
    lora_b = np.ascontiguousarray(lora_b, dtype=np.float32)
    scalings = np.ascontiguousarray(scalings, dtype=np.float32)
    lora_mapping = np.asarray(lora_mapping)

    xT = np.ascontiguousarray(x.T)                                   # [D_IN, N_TOK]
    wT = weight.T                                                    # [D_IN, D_OUT]
    # quad-major W with per-group contiguous (kk, cols) blocks
    w4 = wT.reshape(NQ, KQ, P, D_OUT)                                # [kq,kk,p,o]
    blocks = []
    o0 = 0
    for wdt in GROUPS:
        blk = w4[:, :, :, o0:o0 + wdt * P]                           # [kq,kk,p,w]
        blocks.append(blk.transpose(0, 2, 1, 3).reshape(NQ, P, KQ * wdt * P))
        o0 += wdt * P
    wTr = np.ascontiguousarray(np.concatenate(blocks, axis=2))       # [NQ,P,KQ*D_OUT]

    aT = lora_a.transpose(2, 0, 1).reshape(D_IN, AR)                 # [D_IN,(a r)]
    aTq = np.ascontiguousarray(
        aT.reshape(NQ, KQ, P, AR).transpose(0, 2, 1, 3).reshape(NQ, P, KQ * AR))
    bcat = np.ascontiguousarray(
        lora_b.transpose(0, 2, 1).reshape(AR, D_OUT))                # [(a r), D_OUT]
    # biasr[p, m] = bias[m*128 + p]
    biasr = np.ascontiguousarray(bias.reshape(D_OUT // P, P).T)      # [P, 32]
    # maskT[(a r), n] = scalings[a] * (lora_mapping[n] == a+1)
    ids = np.arange(1, N_ADAPTERS + 1, dtype=lora_mapping.dtype)
    onehot = (lora_mapping[None, :] == ids[:, None]).astype(np.float32)  # [A, N]
    maskT = (onehot * scalings[:, None]).repeat(RANK, axis=0)        # [(a r), N]
    maskT = np.ascontiguousarray(maskT)

    in_maps = []
    for c in range(N_CORES):
        tsl = slice(c * TOK, (c + 1) * TOK)
        in_maps.append({
            "xT": np.ascontiguousarray(xT[:, tsl]),
            "wTr": wTr,
            "aTq": aTq,
            "bcat": bcat,
            "maskT": np.ascontiguousarray(maskT[:, tsl]),
            "biasr": biasr,
        })
    return in_maps


def run(inputs, trace=False):
    if "nc" not in _CACHE:
        _CACHE["nc"] = _build_nc()
    nc = _CACHE["nc"]
    in_maps = _prep_inputs(**inputs)
    res = run_bass_kernel_spmd(
        nc, in_maps, list(range(N_CORES)), trace=trace,
    )
    out = np.concatenate(
        [np.ascontiguousarray(r["outT"].T) for r in res.results], axis=0
    )
    return out, res


def kernel(**inputs) -> np.ndarray:
    out, _ = run(inputs, trace=False)
    return out


# revision 9
# speedup vs baseline: 1.0471x; 1.0121x over previous
"""LiteLinear (dense linear + per-token LoRA adapters) on 8 Trainium2 cores.

Sharding: data-parallel over tokens. Each core computes 1024 tokens:
  out = x @ W^T + bias + per-token LoRA delta.

Device kernel (per core), all matmuls in float32r (full-rate fp32 mode):
  - Computes out^T [D_OUT x TOK]; host transposes back on assembly.
  - Stationary operand = weight sub-chunk [128d x 128o], moving = x^T
    [128d x 512tok]. x^T resident in SBUF (128KB/partition).
  - A_cat^T (the concatenated LoRA down-projections) is prepended to W^T
    as a 33rd output column tile, so h^T = A_cat @ x^T rides the same
    streamed matmul pipeline; its eviction is a DVE multiply with a
    host-built maskT (folds scalings + one-hot) producing hmask^T.
  - The combined [A|W]^T stream is re-laid-out on the host in quad-major
    form: one dma_start per 4 contraction chunks, 2-8KB contiguous lines.
  - o-groups of [4,4,4,4,4,4,4,4,1] x128 tiles (33 total, first group
    includes the A tile); psum = width x 2 token-halves banks.
  - Per-token LoRA delta enters each out-tile as one extra accumulating
    matmul (lhsT=B_cat chunk, rhs=hmask^T); group 0 evicts the h tile
    (producing hmask) before issuing its own deltas.
  - Bias folded into PSUM->SBUF eviction via per-partition
    tensor_scalar_add; output DMAs issue from gpsimd so W prefetch on
    sync never queues behind evictions.
"""

import numpy as np

import sys

if "/opt/trn_rl_repo" not in sys.path:
    sys.path.insert(0, "/opt/trn_rl_repo")

import concourse.bass as bass
import concourse.mybir as mybir
import concourse.tile as tile
from concourse import bacc
from concourse.bass_utils import run_bass_kernel_spmd

N_TOK = 8192
D_IN = 4096
D_OUT = 4096
N_ADAPTERS = 8
RANK = 16
AR = N_ADAPTERS * RANK  # 128
N_CORES = 8
TOK = N_TOK // N_CORES  # 1024 tokens per core

P = 128            # partitions
FREE = 512         # matmul moving free dim (fp32 max, == 1 PSUM bank)
KC = D_IN // P     # 32 contraction chunks
KQ = 4             # k-chunks per quad DMA
NQ = KC // KQ      # 8 quads
TH = TOK // FREE   # 2 token halves
NO = D_OUT // P + 1  # 33 o128-tiles incl. the A tile (index 0)
GROUPS = [4, 4, 4, 4, 4, 4, 4, 4, 1]  # o128-tiles per group (sum 33)

F32 = mybir.dt.float32
F32R = mybir.dt.float32r

_CACHE = {}


def _build_nc():
    nc = bacc.Bacc(None, target_bir_lowering=False, debug=True)

    xT = nc.dram_tensor("xT", [D_IN, TOK], F32R, kind="ExternalInput")
    # quad-major [A|W]: [kq, p, (g kk cols_g)] with per-group contiguous blocks
    wTr = nc.dram_tensor("wTr", [NQ, P, KQ * NO * P], F32R,
                         kind="ExternalInput")
    bcat = nc.dram_tensor("bcat", [AR, D_OUT], F32R, kind="ExternalInput")
    maskT = nc.dram_tensor("maskT", [AR, TOK], F32, kind="ExternalInput")
    biasr = nc.dram_tensor("biasr", [P, D_OUT // P], F32, kind="ExternalInput")
    outT = nc.dram_tensor("outT", [D_OUT, TOK], F32, kind="ExternalOutput")

    def w_quad_src(kq, goff, blk):
        return bass.AP(
            tensor=wTr[:].tensor,
            offset=kq * P * KQ * NO * P + goff,
            ap=[[KQ * NO * P, P], [1, blk]],
        )

    with tile.TileContext(nc) as tc:
        with (
            tc.tile_pool(name="xpool", bufs=1) as xpool,
            tc.tile_pool(name="const", bufs=1) as const,
            tc.tile_pool(name="wpool", bufs=3) as wpool,
            tc.tile_pool(name="opool", bufs=3) as opool,
            tc.tile_pool(name="psum", bufs=8, space="PSUM") as psum,
        ):
            hmask = const.tile([P, TOK], F32R, tag="hmask")
            biasr_sb = const.tile([P, D_OUT // P], F32, tag="biasr")
            maskT_sb = const.tile([P, TOK], F32, tag="maskT")
            bcat_sb = const.tile([P, D_OUT], F32R, tag="bcat")

            xt = []

            def base_loop(g, width, goff, pg, startup):
                """32 k-chunks of base matmuls for one o-group."""
                wt = None
                for k in range(KC):
                    if startup:
                        t = xpool.tile([P, TOK], F32R, tag=f"xt{k}",
                                       name=f"xt{k}")
                        nc.sync.dma_start(out=t[:], in_=xT[k * P:(k + 1) * P, :])
                        xt.append(t)
                    if k % KQ == 0:
                        wt = wpool.tile([P, KQ * width * P], F32R, tag="wt",
                                        name=f"wt{g}_{k}")
                        nc.sync.dma_start(
                            out=wt[:],
                            in_=w_quad_src(k // KQ, goff, KQ * width * P))
                    if startup and k == 1:
                        nc.scalar.dma_start(out=biasr_sb[:], in_=biasr[:, :])
                        nc.scalar.dma_start(out=maskT_sb[:], in_=maskT[:, :])
                    if startup and k == 6:
                        nc.scalar.dma_start(out=bcat_sb[:], in_=bcat[:, :])
                    kk = k % KQ
                    for j in range(width):
                        for th in range(TH):
                            tsl = slice(th * FREE, (th + 1) * FREE)
                            nc.tensor.matmul(
                                pg[j * TH + th][:],
                                wt[:, (kk * width + j) * P:
                                   (kk * width + j + 1) * P],
                                xt[k][:, tsl],
                                start=(k == 0),
                                stop=(k == KC - 1 and g == 0 and j == 0),
                            )

            ooff = 0  # in o128-tiles over the combined [A|W] column space
            for g, width in enumerate(GROUPS):
                pg = [
                    psum.tile([P, FREE], F32, tag="ps", name=f"pg{g}_{i}")
                    for i in range(width * TH)
                ]
                base_loop(g, width, KQ * ooff * P, pg, startup=(g == 0))

                j0 = 0
                if g == 0:
                    # evict the A tile -> hmask (scaled, masked); no delta
                    for th in range(TH):
                        tsl = slice(th * FREE, (th + 1) * FREE)
                        nc.vector.tensor_mul(
                            hmask[:, tsl], pg[th][:], maskT_sb[:, tsl])
                    j0 = 1
                # per-j: delta matmul, then evict+bias, then out DMA
                for j in range(j0, width):
                    om = ooff + j - 1  # real W o128-tile index
                    for th in range(TH):
                        tsl = slice(th * FREE, (th + 1) * FREE)
                        nc.tensor.matmul(
                            pg[j * TH + th][:],
                            bcat_sb[:, om * P:(om + 1) * P],
                            hmask[:, tsl],
                            start=False, stop=True,
                        )
                    ob = opool.tile([P, TOK], F32, tag="ob", name=f"ob_{om}")
                    for th in range(TH):
                        tsl = slice(th * FREE, (th + 1) * FREE)
                        nc.vector.tensor_scalar_add(
                            ob[:, tsl], pg[j * TH + th][:],
                            biasr_sb[:, om:om + 1],
                        )
                    nc.gpsimd.dma_start(
                        out=outT[om * P:(om + 1) * P, :], in_=ob[:]
                    )
                ooff += width

    nc.compile()
    return nc


def _prep_inputs(x, weight, bias, lora_a, lora_b, scalings, lora_mapping):
    x = np.ascontiguousarray(x, dtype=np.float32)
    weight = np.ascontiguousarray(weight, dtype=np.float32)
    bias = np.ascontiguousarray(bias, dtype=np.float32)
    lora_a = np.ascontiguousarray(lora_a, dtype=np.float32)
    lora_b = np.ascontiguousarray(lora_b, dtype=np.float32)
    scalings = np.ascontiguousarray(scalings, dtype=np.float32)
    lora_mapping = np.asarray(lora_mapping)

    xT = np.ascontiguousarray(x.T)                                   # [D_IN, N_TOK]
    aT = lora_a.transpose(2, 0, 1).reshape(D_IN, AR)                 # [D_IN,(a r)]
    awT = np.concatenate([aT, weight.T], axis=1)                     # [D_IN, NO*P]
    # quad-major [A|W] with per-group contiguous (kk, cols) blocks
    w4 = awT.reshape(NQ, KQ, P, NO * P)                              # [kq,kk,p,o]
    blocks = []
    o0 = 0
    for wdt in GROUPS:
        blk = w4[:, :, :, o0:o0 + wdt * P]                           # [kq,kk,p,w]
        blocks.append(blk.transpose(0, 2, 1, 3).reshape(NQ, P, KQ * wdt * P))
        o0 += wdt * P
    wTr = np.ascontiguousarray(np.concatenate(blocks, axis=2))

    bcat = np.ascontiguousarray(
        lora_b.transpose(0, 2, 1).reshape(AR, D_OUT))                # [(a r), D_OUT]
    # biasr[p, m] = bias[m*128 + p]
    biasr = np.ascontiguousarray(bias.reshape(D_OUT // P, P).T)      # [P, 32]
    # maskT[(a r), n] = scalings[a] * (lora_mapping[n] == a+1)
    ids = np.arange(1, N_ADAPTERS + 1, dtype=lora_mapping.dtype)
    onehot = (lora_mapping[None, :] == ids[:, None]).astype(np.float32)  # [A, N]
    maskT = (onehot * scalings[:, None]).repeat(RANK, axis=0)        # [(a r), N]
    maskT = np.ascontiguousarray(maskT)

    in_maps = []
    for c in range(N_CORES):
        tsl = slice(c * TOK, (c + 1) * TOK)
        in_maps.append({
            "xT": np.ascontiguousarray(xT[:, tsl]),
            "wTr": wTr,
            "bcat": bcat,
            "maskT": np.ascontiguousarray(maskT[:, tsl]),
            "biasr": biasr,
        })
    return in_maps


def run(inputs, trace=False):
    if "nc" not in _CACHE:
        _CACHE["nc"] = _build_nc()
    nc = _CACHE["nc"]
    in_maps = _prep_inputs(**inputs)
    res = run_bass_kernel_spmd(
        nc, in_maps, list(range(N_CORES)), trace=trace,
    )
    out = np.concatenate(
        [np.ascontiguousarray(r["outT"].T) for r in res.results], axis=0
    )
    return out, res


def kernel(**inputs) -> np.ndarray:
    out, _ = run(inputs, trace=False)
    return out
